# revision 35
# baseline (speedup 1.0000x reference)
"""V5 Trainium2 Bass kernel for nn_Attention (B=4, N=2048, H=12, D=64).

Sharding: 8 cores = 4 batches x 2 head-groups of 6 heads. Per core: fused
qkv-projection + attention + output-projection, bf16 matmuls / fp32 psum.

Structure (V2 base): AV matmuls interleaved into the QK^T/exp chunk stream;
score chunks CH=2 ([128,2,512] psum, 2 banks); qkv/output projections emitted
as "filler" groups spread across chunk steps. Banks: scores 2x2, AV 2, misc 2.

V4/V5 changes (measured on HW via reps-differencing, body-iter 471us -> ~310us):
- Normalize chain: DVE `reciprocal` is 8 cyc/elem (~4.3us per [1,512]!);
  replaced with bit-trick seed + 1 Newton step (plain DVE TT/TS ops, max rel
  err 2.6e-3), and the PSUM copy + multiply fused into one
  scalar_tensor_tensor that reads the AV psum directly.  [-60us]
- AV split 10 bf16 / 6 fp8 kv-blocks: high blocks use fp8e4 DoubleRow
  (2 kv blocks per matmul; exp written as fp8 with bias -2). Full-fp8 fails
  the 2e-2 gate (2.5e-2); this split measures 1.59e-2.  [-25us]
- v_group inverted: x-block stationary, wv streams 384 wide (one matmul per
  contraction block for all 6 heads instead of three 128-wide ones). [-15us]

Known-flat: 2-strip/2-half weight-reuse restructures of qk_group/proj_group
regressed (no LDWEIGHTS dedup win; interleaved 2-bank accumulation hurts).
QK^T 64-row tile_position head-pairs DO run concurrently on HW.
"""

import sys

if "/opt/trn_rl_repo" not in sys.path:
    sys.path.insert(0, "/opt/trn_rl_repo")

import numpy as np
import ml_dtypes

import concourse.bacc as bacc
import concourse.mybir as mybir
import concourse.tile as tile

FP32 = mybir.dt.float32
BF16 = mybir.dt.bfloat16
FP8 = mybir.dt.float8e4
U32 = mybir.dt.uint32
AF = mybir.ActivationFunctionType
ALU = mybir.AluOpType
DR = mybir.MatmulPerfMode.DoubleRow
EXP_BIAS = -2.0  # exp(s-2): keeps values <=~40, inside fp8e4 range
RECIP_MAGIC = 0x7EF311C3  # fp32 reciprocal bit-trick seed (1 Newton iter after)
NHI = 6  # kv blocks (of 16) routed through fp8 DoubleRow AV; rest bf16

DIM = 768
HEAD_DIM = 64
SCALE = HEAD_DIM ** -0.5
B, N = 4, 2048
HG = 6
CC = DIM // 128
PAIRS = HG // 2
S = N // 512
J = N // 128
CH = 2                      # kv blocks per score chunk
NCHUNK = J // CH            # 8 chunks per (head, strip)

_CACHED = {}


def build_core_program(reps=0, ablate=(), bodies=1):
    nc = bacc.Bacc("TRN2", debug=False, target_bir_lowering=False, num_devices=1)

    xt_d = nc.dram_tensor("xt", [DIM, N], BF16, kind="ExternalInput")
    wqk_d = nc.dram_tensor("wqk", [DIM, DIM], BF16, kind="ExternalInput")
    wv_d = nc.dram_tensor("wv", [DIM, HG * 64], BF16, kind="ExternalInput")
    wp_d = nc.dram_tensor("wp", [HG * 64, DIM], BF16, kind="ExternalInput")
    y_d = nc.dram_tensor("y", [N, DIM], FP32, kind="ExternalOutput")

    with tile.TileContext(nc) as tc:
        with (
            tc.tile_pool(name="persist", bufs=1) as persist,
            tc.tile_pool(name="exps", bufs=22) as exps_pool,
            tc.tile_pool(name="attnt", bufs=5) as attnt_pool,
            tc.tile_pool(name="small", bufs=4) as small_pool,
            tc.tile_pool(name="ysb", bufs=3) as y_pool,
            tc.tile_pool(name="ps_score", bufs=2, space="PSUM") as ps_score,
            tc.tile_pool(name="ps_av", bufs=2, space="PSUM") as ps_av,
            tc.tile_pool(name="ps_misc", bufs=2, space="PSUM") as ps_misc,
        ):
            JLO = J - NHI       # low kv blocks: bf16 AV
            CLO = JLO // CH     # chunks below this index are bf16

            xT = persist.tile([128, CC, N], BF16)
            wqk = persist.tile([128, CC, DIM], BF16)
            wv = persist.tile([128, CC, HG * 64], BF16)
            wp = persist.tile([128, PAIRS, DIM], BF16)
            qkT = persist.tile([128, CC, N], BF16)
            vlo = persist.tile([128, JLO, HG, 65], BF16)
            # high blocks in fp8, paired for DoubleRow: [key, pair, plane, head, d]
            # d padded 65->80 so the plane stride (HG*80 = 480B) is 16B-aligned
            vhi = persist.tile([128, NHI // 2, 2, HG, 80], FP8)
            ebias = persist.tile([128, 1], FP32)
            nc.vector.memset(ebias, EXP_BIAS)
            magic = persist.tile([1, 512], U32)
            nc.vector.memset(magic, RECIP_MAGIC)

            xt_r = xt_d.ap().rearrange("(o p) n -> p o n", p=128)
            wqk_r = wqk_d.ap().rearrange("(o p) n -> p o n", p=128)
            wv_r = wv_d.ap().rearrange("(o p) n -> p o n", p=128)
            for cc in range(CC):
                nc.sync.dma_start(out=wqk[:, cc], in_=wqk_r[:, cc])
                nc.sync.dma_start(out=xT[:, cc], in_=xt_r[:, cc])
                nc.sync.dma_start(out=wv[:, cc], in_=wv_r[:, cc])
            nc.sync.dma_start(out=wp, in_=wp_d.ap().rearrange("(o p) n -> p o n", p=128))
            nc.vector.memset(vlo, 1.0)
            nc.vector.memset(vhi, 1.0)

            # ---------- filler groups (each: one psum-group of work) ----------

            def qk_group(ot, s):
                ps = ps_misc.tile([128, 512], FP32, tag="m")
                for cc in range(CC):
                    nc.tensor.matmul(
                        ps,
                        wqk[:, cc, ot * 128 : ot * 128 + 128],
                        xT[:, cc, s * 512 : s * 512 + 512],
                        start=(cc == 0), stop=(cc == CC - 1),
                    )
                nc.vector.tensor_copy(out=qkT[:, ot, s * 512 : s * 512 + 512], in_=ps)

            def v_group(nt):
                # all 6 heads at once: x-block stationary, wv streams 384 wide
                psv = ps_misc.tile([128, 512], FP32, tag="m")
                for cc in range(CC):
                    nc.tensor.matmul(
                        psv[:, 0 : HG * 64],
                        xT[:, cc, nt * 128 : nt * 128 + 128],
                        wv[:, cc, :],
                        start=(cc == 0), stop=(cc == CC - 1),
                    )
                for h in range(HG):
                    if nt < JLO:
                        vdst = vlo[:, nt, h, 0:64]
                    else:
                        m = nt - JLO
                        vdst = vhi[:, m // 2, m % 2, h, 0:64]
                    nc.vector.tensor_copy(
                        out=vdst,
                        in_=psv[:, h * 64 : h * 64 + 64],
                    )

            def proj_group(s, attnT, nt, og, ow, ysb):
                psy = ps_misc.tile([128, 512], FP32, tag="m")
                for cc in range(PAIRS):
                    nc.tensor.matmul(
                        psy[:, 0:ow],
                        attnT[:, cc, nt * 128 : nt * 128 + 128],
                        wp[:, cc, og : og + ow],
                        start=(cc == 0), stop=(cc == PAIRS - 1),
                    )
                nc.vector.tensor_copy(out=ysb[:, og : og + ow], in_=psy[:, 0:ow])
                if og == 512:
                    row = s * 512 + nt * 128
                    nc.sync.dma_start(out=y_d.ap()[row : row + 128, :], in_=ysb)

            def qkv_pair_fillers(p):
                out = []
                for ot in (p, PAIRS + p):
                    for s in range(S):
                        out.append(lambda ot=ot, s=s: qk_group(ot, s))
                return out

            def proj_fillers(s, attnT):
                if "fill" in ablate:
                    return []
                out = []
                for nt in range(4):
                    ysb = y_pool.tile([128, DIM], FP32, tag="y")
                    for og, ow in ((0, 512), (512, 256)):
                        out.append(
                            lambda nt=nt, og=og, ow=ow, ysb=ysb:
                                proj_group(s, attnT, nt, og, ow, ysb)
                        )
                return out

            def body():
                # lead-in: only qT/kT of pair 0; its v groups trickle into
                # slot (0,0)'s chunk stream (ACT starts ~18us earlier)
                for ot in (0, PAIRS):
                    for s in range(S):
                        qk_group(ot, s)

                filler = []
                attnT_tiles = {}
                for hp in range(PAIRS):
                    if hp + 1 < PAIRS:
                        filler.extend(qkv_pair_fillers(hp + 1))
                    for s in range(S):
                        if hp == 0:
                            at = attnt_pool.tile(
                                [128, PAIRS, 512], BF16, tag="attnT",
                                name=f"attnT{s}",
                            )
                            attnT_tiles[s] = at
                        attnT = attnT_tiles[s]

                        pav = {}
                        for h2 in range(2):
                            pav[h2] = ps_av.tile(
                                [128, 512], FP32, tag="av", name=f"pav{h2}"
                            )

                        expS = {0: [None] * NCHUNK, 1: [None] * NCHUNK}

                        def emit_qkt_exp(c, s=s, hp=hp, expS=expS):
                            for h2, base in ((0, 0), (1, 64)):
                                pss = ps_score.tile(
                                    [128, CH, 512], FP32, tag="sc"
                                )
                                if "qkt" in ablate and "exp" not in ablate:
                                    nc.vector.memset(pss[:, 0, 0:2], 0.0)
                                for jj in range(CH):
                                    j = c * CH + jj
                                    if "qkt" in ablate:
                                        continue
                                    nc.tensor.matmul(
                                        pss[:, jj, :],
                                        qkT[base : base + 64, PAIRS + hp,
                                            j * 128 : j * 128 + 128],
                                        qkT[base : base + 64, hp,
                                            s * 512 : s * 512 + 512],
                                        start=True, stop=True,
                                        tile_position=(base, 0),
                                    )
                                et = exps_pool.tile(
                                    [128, CH, 512],
                                    BF16 if c < CLO else FP8, tag="e",
                                )
                                if "exp" not in ablate:
                                    nc.scalar.activation(
                                        out=et, in_=pss, func=AF.Exp,
                                        bias=ebias[:, :],
                                    )
                                else:
                                    nc.vector.memset(et[:, 0, 0:4], 1.0)
                                expS[h2][c] = et

                        def emit_av(c, hp=hp, pav=pav, expS=expS):
                            if "av" in ablate:
                                return
                            for h2 in range(2):
                                h = 2 * hp + h2
                                et = expS[h2][c]
                                if c < CLO:
                                    for jj in range(CH):
                                        j = c * CH + jj
                                        nc.tensor.matmul(
                                            pav[h2][0:65, :],
                                            vlo[:, j, h, :],
                                            et[:, jj, :],
                                            start=(j == 0), stop=False,
                                            skip_group_check=True,
                                        )
                                else:
                                    m = c - CLO
                                    nc.tensor.matmul(
                                        pav[h2][0:65, :],
                                        vhi[:, m, :, h, 0:65],
                                        et[:, :, :],
                                        start=False, stop=(c == NCHUNK - 1),
                                        perf_mode=DR,
                                        skip_group_check=True,
                                    )

                        first_slot = hp == 0 and s == 0
                        for c in range(NCHUNK):
                            emit_qkt_exp(c)
                            if c > 0:
                                if first_slot and "fill" not in ablate:
                                    v_group(2 * (c - 1))
                                    v_group(2 * (c - 1) + 1)
                                emit_av(c - 1)
                            if filler and not first_slot:
                                filler.pop(0)()
                        if first_slot and "fill" not in ablate:
                            v_group(14)
                            v_group(15)
                        emit_av(NCHUNK - 1)

                        if "av" in ablate:
                            for h2 in range(2):
                                nc.vector.memset(pav[h2][0:1, 0:2], 1.0)

                        # normalize: 1/denom via bit-trick + 1 Newton step on
                        # DVE (reciprocal op is 8 cyc/elem - far too slow),
                        # then Pool broadcast, then one fused PSUM-read
                        # multiply into attnT
                        for h2 in range(2):
                            if "div" in ablate:
                                nc.vector.memset(
                                    attnT[h2 * 64 : h2 * 64 + 64, hp, 0:2], 0.5
                                )
                                continue
                            den = pav[h2][64:65, :]
                            x0 = small_pool.tile([1, 512], FP32, tag="x0")
                            nc.vector.tensor_tensor(
                                x0[:, :].bitcast(U32), magic,
                                den.bitcast(U32), ALU.subtract,
                            )
                            t = small_pool.tile([1, 512], FP32, tag="t")
                            nc.vector.tensor_tensor(t, den, x0, ALU.mult)
                            u = small_pool.tile([1, 512], FP32, tag="u")
                            nc.vector.tensor_scalar(
                                out=u, in0=t, scalar1=-1.0, scalar2=2.0,
                                op0=ALU.mult, op1=ALU.add,
                            )
                            x1 = small_pool.tile([1, 512], FP32, tag="x1")
                            nc.vector.tensor_tensor(x1, x0, u, ALU.mult)
                            rb = small_pool.tile([64, 512], FP32, tag="rb")
                            nc.gpsimd.partition_broadcast(rb, x1, channels=64)
                            nc.vector.scalar_tensor_tensor(
                                out=attnT[h2 * 64 : h2 * 64 + 64, hp, :],
                                in0=pav[h2][0:64, :],
                                scalar=1.0,
                                in1=rb,
                                op0=ALU.mult,
                                op1=ALU.mult,
                            )

                        if hp == PAIRS - 1:
                            filler.extend(
                                proj_fillers(s, attnT_tiles.pop(s))
                            )

                # drain remaining fillers (tail projections)
                for f in filler:
                    f()

            from contextlib import nullcontext
            with (tc.For_i(0, reps, 1) if reps else nullcontext()):
                for _ in range(bodies):
                    body()

    nc.compile()
    return nc


def _host_prep(x, w_qkv, w_proj):
    bf16 = ml_dtypes.bfloat16
    in_maps = []
    for c in range(8):
        b, hg = c // 2, c % 2
        r0 = 384 * hg
        wq = w_qkv[r0 : r0 + 384] * SCALE
        wk = w_qkv[768 + r0 : 768 + r0 + 384]
        wvv = w_qkv[1536 + r0 : 1536 + r0 + 384]
        wqk = np.concatenate([wq, wk], axis=0)
        in_maps.append({
            "xt": np.ascontiguousarray(x[b].T).astype(bf16),
            "wqk": np.ascontiguousarray(wqk.T).astype(bf16),
            "wv": np.ascontiguousarray(wvv.T).astype(bf16),
            "wp": np.ascontiguousarray(w_proj[:, r0 : r0 + 384].T).astype(bf16),
        })
    return in_maps


def _get_fn():
    if "fn" in _CACHED:
        return _CACHED["fn"]

    import jax
    from jax.sharding import Mesh, PartitionSpec
    from jax.experimental.shard_map import shard_map
    from concourse import bass2jax
    from concourse.bass2jax import _bass_exec_p, install_neuronx_cc_hook

    install_neuronx_cc_hook()
    nc = build_core_program()

    in_names = ["xt", "wqk", "wv", "wp"]
    out_avals = [jax.core.ShapedArray((N, DIM), np.float32)]
    partition_name = nc.partition_id_tensor.name if nc.partition_id_tensor else None

    def _body(xt, wqk, wvv, wp, yzero):
        operands = [xt, wqk, wvv, wp, yzero]
        names = in_names + ["y"]
        if nc.dbg_addr is not None:
            operands.append(np.zeros((1, 2), np.uint32))
            names.append(nc.dbg_addr.name)
        if partition_name is not None:
            operands.append(bass2jax.partition_id_tensor())
            names.append(partition_name)
        outs = _bass_exec_p.bind(
            *operands,
            out_avals=tuple(out_avals),
            in_names=tuple(names),
            out_names=("y",),
            lowering_input_output_aliases=(),
            sim_require_finite=True,
            sim_require_nnan=True,
            nc=nc,
        )
        return outs[0]

    devices = jax.devices()[:8]
    mesh = Mesh(np.asarray(devices), ("core",))
    fn = jax.jit(
        shard_map(
            _body, mesh=mesh,
            in_specs=(PartitionSpec("core"),) * 5,
            out_specs=PartitionSpec("core"),
            check_rep=False,
        ),
        keep_unused=True,
    )
    _CACHED["fn"] = fn
    return fn


def _run(in_maps):
    import jax

    fn = _get_fn()
    concat_in = [
        np.concatenate([m[name] for m in in_maps], axis=0)
        for name in ["xt", "wqk", "wv", "wp"]
    ]
    yzero = np.zeros((8 * N, DIM), np.float32)
    out = jax.block_until_ready(fn(*concat_in, yzero))
    return np.asarray(out).reshape(8, N, DIM)


def kernel(x, w_qkv, w_proj, b_proj):
    x = np.asarray(x, dtype=np.float32)
    w_qkv = np.asarray(w_qkv, dtype=np.float32)
    w_proj = np.asarray(w_proj, dtype=np.float32)
    b_proj = np.asarray(b_proj, dtype=np.float32)

    in_maps = _host_prep(x, w_qkv, w_proj)
    parts = _run(in_maps)

    y = np.empty((B, N, DIM), dtype=np.float32)
    for b in range(B):
        y[b] = parts[2 * b] + parts[2 * b + 1] + b_proj
    return y



# revision 57
# speedup vs baseline: 1.1650x; 1.1650x over previous
"""V10 Trainium2 Bass kernel for nn_Attention (B=4, N=2048, H=12, D=64).

Sharding: 8 cores = 4 batches x 2 head-groups of 6 heads. Per core: fused
qkv-projection + attention + output-projection, bf16 matmuls / fp32 psum.

Structure (V2 base): AV matmuls interleaved into the QK^T/exp chunk stream;
score chunks CH=2 ([128,2,512] psum, 2 banks); qkv/output projections emitted
as "filler" groups spread across chunk steps. Banks: scores 2x2, AV 2, misc 2.

V4/V5 changes (measured on HW via reps-differencing, body-iter 471us -> ~310us):
- Normalize chain: DVE `reciprocal` is 8 cyc/elem (~4.3us per [1,512]!);
  replaced with bit-trick seed + 1 Newton step (plain DVE TT/TS ops, max rel
  err 2.6e-3), and the PSUM copy + multiply fused into one
  scalar_tensor_tensor that reads the AV psum directly.  [-60us]
- AV split 10 bf16 / 6 fp8 kv-blocks: high blocks use fp8e4 DoubleRow
  (2 kv blocks per matmul; exp written as fp8 with bias -2). Full-fp8 fails
  the 2e-2 gate (2.5e-2); this split measures 1.59e-2.  [-25us]
- v_group inverted: x-block stationary, wv streams 384 wide (one matmul per
  contraction block for all 6 heads instead of three 128-wide ones). [-15us]

V8/V10: AV matmuls lag their exp by 2 chunks (never stall the PE FIFO on
ACT), and the slot tail (last two AV groups + both normalize chains, ~9us of serial
PE-then-DVE work) is deferred as ONE closure and emitted right after the
NEXT slot's first QK chunk. The next exp is then never queued behind the
boundary work, and emission stays read-before-write on the reused ps_av
banks (finer-grained deferral of individual chain ops corrupts results -
see make_chain comment). Measured -12% body-iter vs V5 in an A/B window.

Known-flat: 2-strip/2-half weight-reuse restructures of qk_group/proj_group
regressed (no LDWEIGHTS dedup win; interleaved 2-bank accumulation hurts).
QK^T 64-row tile_position head-pairs DO run concurrently on HW.
"""

import sys

if "/opt/trn_rl_repo" not in sys.path:
    sys.path.insert(0, "/opt/trn_rl_repo")

import numpy as np
import ml_dtypes

import concourse.bacc as bacc
import concourse.mybir as mybir
import concourse.tile as tile

FP32 = mybir.dt.float32
BF16 = mybir.dt.bfloat16
FP8 = mybir.dt.float8e4
U32 = mybir.dt.uint32
AF = mybir.ActivationFunctionType
ALU = mybir.AluOpType
DR = mybir.MatmulPerfMode.DoubleRow
EXP_BIAS = -2.0  # exp(s-2): keeps values <=~40, inside fp8e4 range
RECIP_MAGIC = 0x7EF311C3  # fp32 reciprocal bit-trick seed (1 Newton iter after)
NHI = 6  # kv blocks (of 16) routed through fp8 DoubleRow AV; rest bf16

DIM = 768
HEAD_DIM = 64
SCALE = HEAD_DIM ** -0.5
B, N = 4, 2048
HG = 6
CC = DIM // 128
PAIRS = HG // 2
S = N // 512
J = N // 128
CH = 2                      # kv blocks per score chunk
NCHUNK = J // CH            # 8 chunks per (head, strip)

_CACHED = {}


def build_core_program(reps=0, ablate=(), bodies=1):
    nc = bacc.Bacc("TRN2", debug=False, target_bir_lowering=False, num_devices=1)

    xt_d = nc.dram_tensor("xt", [DIM, N], BF16, kind="ExternalInput")
    wqk_d = nc.dram_tensor("wqk", [DIM, DIM], BF16, kind="ExternalInput")
    wv_d = nc.dram_tensor("wv", [DIM, HG * 64], BF16, kind="ExternalInput")
    wp_d = nc.dram_tensor("wp", [HG * 64, DIM], BF16, kind="ExternalInput")
    y_d = nc.dram_tensor("y", [N, DIM], FP32, kind="ExternalOutput")

    with tile.TileContext(nc) as tc:
        with (
            tc.tile_pool(name="persist", bufs=1) as persist,
            tc.tile_pool(name="exps", bufs=22) as exps_pool,
            tc.tile_pool(name="attnt", bufs=5) as attnt_pool,
            tc.tile_pool(name="small", bufs=4) as small_pool,
            tc.tile_pool(name="ysb", bufs=3) as y_pool,
            tc.tile_pool(name="ps_score", bufs=2, space="PSUM") as ps_score,
            tc.tile_pool(name="ps_av", bufs=2, space="PSUM") as ps_av,
            tc.tile_pool(name="ps_misc", bufs=2, space="PSUM") as ps_misc,
        ):
            JLO = J - NHI       # low kv blocks: bf16 AV
            CLO = JLO // CH     # chunks below this index are bf16

            xT = persist.tile([128, CC, N], BF16)
            wqk = persist.tile([128, CC, DIM], BF16)
            wv = persist.tile([128, CC, HG * 64], BF16)
            wp = persist.tile([128, PAIRS, DIM], BF16)
            qkT = persist.tile([128, CC, N], BF16)
            vlo = persist.tile([128, JLO, HG, 65], BF16)
            # high blocks in fp8, paired for DoubleRow: [key, pair, plane, head, d]
            # d padded 65->80 so the plane stride (HG*80 = 480B) is 16B-aligned
            vhi = persist.tile([128, NHI // 2, 2, HG, 80], FP8)
            ebias = persist.tile([128, 1], FP32)
            nc.vector.memset(ebias, EXP_BIAS)
            magic = persist.tile([1, 512], U32)
            nc.vector.memset(magic, RECIP_MAGIC)

            xt_r = xt_d.ap().rearrange("(o p) n -> p o n", p=128)
            wqk_r = wqk_d.ap().rearrange("(o p) n -> p o n", p=128)
            wv_r = wv_d.ap().rearrange("(o p) n -> p o n", p=128)
            for cc in range(CC):
                nc.sync.dma_start(out=wqk[:, cc], in_=wqk_r[:, cc])
                nc.sync.dma_start(out=xT[:, cc], in_=xt_r[:, cc])
                nc.sync.dma_start(out=wv[:, cc], in_=wv_r[:, cc])
            nc.sync.dma_start(out=wp, in_=wp_d.ap().rearrange("(o p) n -> p o n", p=128))
            nc.vector.memset(vlo, 1.0)
            nc.vector.memset(vhi, 1.0)

            # ---------- filler groups (each: one psum-group of work) ----------

            def qk_group(ot, s):
                ps = ps_misc.tile([128, 512], FP32, tag="m")
                for cc in range(CC):
                    nc.tensor.matmul(
                        ps,
                        wqk[:, cc, ot * 128 : ot * 128 + 128],
                        xT[:, cc, s * 512 : s * 512 + 512],
                        start=(cc == 0), stop=(cc == CC - 1),
                    )
                nc.vector.tensor_copy(out=qkT[:, ot, s * 512 : s * 512 + 512], in_=ps)

            def v_group(nt):
                # all 6 heads at once: x-block stationary, wv streams 384 wide
                psv = ps_misc.tile([128, 512], FP32, tag="m")
                for cc in range(CC):
                    nc.tensor.matmul(
                        psv[:, 0 : HG * 64],
                        xT[:, cc, nt * 128 : nt * 128 + 128],
                        wv[:, cc, :],
                        start=(cc == 0), stop=(cc == CC - 1),
                    )
                for h in range(HG):
                    if nt < JLO:
                        vdst = vlo[:, nt, h, 0:64]
                    else:
                        m = nt - JLO
                        vdst = vhi[:, m // 2, m % 2, h, 0:64]
                    nc.vector.tensor_copy(
                        out=vdst,
                        in_=psv[:, h * 64 : h * 64 + 64],
                    )

            def proj_group(s, attnT, nt, og, ow, ysb):
                psy = ps_misc.tile([128, 512], FP32, tag="m")
                for cc in range(PAIRS):
                    nc.tensor.matmul(
                        psy[:, 0:ow],
                        attnT[:, cc, nt * 128 : nt * 128 + 128],
                        wp[:, cc, og : og + ow],
                        start=(cc == 0), stop=(cc == PAIRS - 1),
                    )
                nc.vector.tensor_copy(out=ysb[:, og : og + ow], in_=psy[:, 0:ow])
                if og == 512:
                    row = s * 512 + nt * 128
                    nc.sync.dma_start(out=y_d.ap()[row : row + 128, :], in_=ysb)

            def qkv_pair_fillers(p):
                out = []
                for ot in (p, PAIRS + p):
                    for s in range(S):
                        out.append(lambda ot=ot, s=s: qk_group(ot, s))
                return out

            def proj_fillers(s, attnT):
                if "fill" in ablate:
                    return []
                out = []
                for nt in range(4):
                    ysb = y_pool.tile([128, DIM], FP32, tag="y")
                    for og, ow in ((0, 512), (512, 256)):
                        out.append(
                            lambda nt=nt, og=og, ow=ow, ysb=ysb:
                                proj_group(s, attnT, nt, og, ow, ysb)
                        )
                return out

            def body():
                # lead-in: only qT/kT of pair 0; its v groups trickle into
                # slot (0,0)'s chunk stream (ACT starts ~18us earlier)
                for ot in (0, PAIRS):
                    for s in range(S):
                        qk_group(ot, s)

                filler = []
                pending = []   # deferred normalize-chain ops (DVE/Pool)
                attnT_tiles = {}

                def make_chain(pav, attnT, hp, h2):
                    # normalize: 1/denom via bit-trick + 1 Newton step on
                    # DVE (reciprocal op is 8 cyc/elem - far too slow), then
                    # Pool broadcast + one fused PSUM-read multiply into
                    # attnT. All ops run inline at slot end: deferring any of
                    # them into the next slot's chunk stream (to unblock the
                    # DVE queue) corrupts results - ps_av's 2 buffers are
                    # rewritten next slot, and even SBUF-only deferred ops
                    # came back with stale values. Returns no deferred ops.
                    if "div" in ablate:
                        return [lambda: nc.vector.memset(
                            attnT[h2 * 64 : h2 * 64 + 64, hp, 0:2], 0.5)]
                    den = pav[h2][64:65, :]
                    x0 = small_pool.tile([1, 512], FP32, tag="x0")
                    nc.vector.tensor_tensor(
                        x0[:, :].bitcast(U32), magic,
                        den.bitcast(U32), ALU.subtract,
                    )
                    t = small_pool.tile([1, 512], FP32, tag="t")
                    nc.vector.tensor_tensor(t, den, x0, ALU.mult)
                    u = small_pool.tile([1, 512], FP32, tag="u")
                    nc.vector.tensor_scalar(
                        out=u, in0=t, scalar1=-1.0, scalar2=2.0,
                        op0=ALU.mult, op1=ALU.add,
                    )
                    x1 = small_pool.tile([1, 512], FP32, tag="x1")
                    nc.vector.tensor_tensor(x1, x0, u, ALU.mult)
                    rb = small_pool.tile([64, 512], FP32, tag="rb")
                    nc.gpsimd.partition_broadcast(rb, x1, channels=64)
                    nc.vector.scalar_tensor_tensor(
                        out=attnT[h2 * 64 : h2 * 64 + 64, hp, :],
                        in0=pav[h2][0:64, :],
                        scalar=1.0,
                        in1=rb,
                        op0=ALU.mult,
                        op1=ALU.mult,
                    )
                    return []

                for hp in range(PAIRS):
                    if hp + 1 < PAIRS:
                        filler.extend(qkv_pair_fillers(hp + 1))
                    for s in range(S):
                        if hp == 0:
                            at = attnt_pool.tile(
                                [128, PAIRS, 512], BF16, tag="attnT",
                                name=f"attnT{s}",
                            )
                            attnT_tiles[s] = at
                        attnT = attnT_tiles[s]

                        pav = {}
                        for h2 in range(2):
                            pav[h2] = ps_av.tile(
                                [128, 512], FP32, tag="av", name=f"pav{h2}"
                            )

                        expS = {0: [None] * NCHUNK, 1: [None] * NCHUNK}

                        def emit_qkt_exp(c, s=s, hp=hp, expS=expS):
                            for h2, base in ((0, 0), (1, 64)):
                                pss = ps_score.tile(
                                    [128, CH, 512], FP32, tag="sc"
                                )
                                if "qkt" in ablate and "exp" not in ablate:
                                    nc.vector.memset(pss[:, 0, 0:2], 0.0)
                                for jj in range(CH):
                                    j = c * CH + jj
                                    if "qkt" in ablate:
                                        continue
                                    nc.tensor.matmul(
                                        pss[:, jj, :],
                                        qkT[base : base + 64, PAIRS + hp,
                                            j * 128 : j * 128 + 128],
                                        qkT[base : base + 64, hp,
                                            s * 512 : s * 512 + 512],
                                        start=True, stop=True,
                                        tile_position=(base, 0),
                                    )
                                et = exps_pool.tile(
                                    [128, CH, 512],
                                    BF16 if c < CLO else FP8, tag="e",
                                )
                                if "exp" not in ablate:
                                    nc.scalar.activation(
                                        out=et, in_=pss, func=AF.Exp,
                                        bias=ebias[:, :],
                                    )
                                else:
                                    nc.vector.memset(et[:, 0, 0:4], 1.0)
                                expS[h2][c] = et

                        def emit_av(c, hp=hp, pav=pav, expS=expS):
                            if "av" in ablate:
                                return
                            for h2 in range(2):
                                h = 2 * hp + h2
                                et = expS[h2][c]
                                if c < CLO:
                                    for jj in range(CH):
                                        j = c * CH + jj
                                        nc.tensor.matmul(
                                            pav[h2][0:65, :],
                                            vlo[:, j, h, :],
                                            et[:, jj, :],
                                            start=(j == 0), stop=False,
                                            skip_group_check=True,
                                        )
                                else:
                                    m = c - CLO
                                    nc.tensor.matmul(
                                        pav[h2][0:65, :],
                                        vhi[:, m, :, h, 0:65],
                                        et[:, :, :],
                                        start=False, stop=(c == NCHUNK - 1),
                                        perf_mode=DR,
                                        skip_group_check=True,
                                    )

                        first_slot = hp == 0 and s == 0
                        for c in range(NCHUNK):
                            emit_qkt_exp(c)
                            if c == 0 and pending:
                                # previous slot's tail (its last two AV
                                # groups + normalize chains) runs after this
                                # slot's first QK chunk: the next exp is
                                # never delayed behind boundary work, and the
                                # chains' pav reads and attnT writes are
                                # emitted before this slot's first pav write
                                # (c==2) and before any proj filler pop
                                pending.pop(0)()
                            if c > 0 and first_slot and "fill" not in ablate:
                                v_group(2 * (c - 1))
                                v_group(2 * (c - 1) + 1)
                            if c > 1:
                                # AV lags 2 chunks: by the time an AV matmul
                                # reaches the PE FIFO head its exp input is
                                # long finished, so it never stalls the queue
                                emit_av(c - 2)
                            if filler and not first_slot:
                                filler.pop(0)()
                        if first_slot and "fill" not in ablate:
                            v_group(14)
                            v_group(15)

                        def slot_tail(pav=pav, attnT=attnT, hp=hp,
                                      emit_av=emit_av):
                            emit_av(NCHUNK - 2)
                            emit_av(NCHUNK - 1)
                            if "av" in ablate:
                                for h2 in range(2):
                                    nc.vector.memset(pav[h2][0:1, 0:2], 1.0)
                            for h2 in range(2):
                                make_chain(pav, attnT, hp, h2)

                        pending.append(slot_tail)

                        if hp == PAIRS - 1:
                            filler.extend(
                                proj_fillers(s, attnT_tiles.pop(s))
                            )

                # drain the last slot's tail, then remaining fillers (tail
                # projections - they depend on the last chains' attnT)
                for op in pending:
                    op()
                for f in filler:
                    f()

            from contextlib import nullcontext
            with (tc.For_i(0, reps, 1) if reps else nullcontext()):
                for _ in range(bodies):
                    body()

    nc.compile()
    return nc


def _host_prep(x, w_qkv, w_proj):
    bf16 = ml_dtypes.bfloat16
    in_maps = []
    for c in range(8):
        b, hg = c // 2, c % 2
        r0 = 384 * hg
        wq = w_qkv[r0 : r0 + 384] * SCALE
        wk = w_qkv[768 + r0 : 768 + r0 + 384]
        wvv = w_qkv[1536 + r0 : 1536 + r0 + 384]
        wqk = np.concatenate([wq, wk], axis=0)
        in_maps.append({
            "xt": np.ascontiguousarray(x[b].T).astype(bf16),
            "wqk": np.ascontiguousarray(wqk.T).astype(bf16),
            "wv": np.ascontiguousarray(wvv.T).astype(bf16),
            "wp": np.ascontiguousarray(w_proj[:, r0 : r0 + 384].T).astype(bf16),
        })
    return in_maps


def _get_fn():
    if "fn" in _CACHED:
        return _CACHED["fn"]

    import jax
    from jax.sharding import Mesh, PartitionSpec
    from jax.experimental.shard_map import shard_map
    from concourse import bass2jax
    from concourse.bass2jax import _bass_exec_p, install_neuronx_cc_hook

    install_neuronx_cc_hook()
    nc = build_core_program()

    in_names = ["xt", "wqk", "wv", "wp"]
    out_avals = [jax.core.ShapedArray((N, DIM), np.float32)]
    partition_name = nc.partition_id_tensor.name if nc.partition_id_tensor else None

    def _body(xt, wqk, wvv, wp, yzero):
        operands = [xt, wqk, wvv, wp, yzero]
        names = in_names + ["y"]
        if nc.dbg_addr is not None:
            operands.append(np.zeros((1, 2), np.uint32))
            names.append(nc.dbg_addr.name)
        if partition_name is not None:
            operands.append(bass2jax.partition_id_tensor())
            names.append(partition_name)
        outs = _bass_exec_p.bind(
            *operands,
            out_avals=tuple(out_avals),
            in_names=tuple(names),
            out_names=("y",),
            lowering_input_output_aliases=(),
            sim_require_finite=True,
            sim_require_nnan=True,
            nc=nc,
        )
        return outs[0]

    devices = jax.devices()[:8]
    mesh = Mesh(np.asarray(devices), ("core",))
    fn = jax.jit(
        shard_map(
            _body, mesh=mesh,
            in_specs=(PartitionSpec("core"),) * 5,
            out_specs=PartitionSpec("core"),
            check_rep=False,
        ),
        keep_unused=True,
    )
    _CACHED["fn"] = fn
    return fn


def _run(in_maps):
    import jax

    fn = _get_fn()
    concat_in = [
        np.concatenate([m[name] for m in in_maps], axis=0)
        for name in ["xt", "wqk", "wv", "wp"]
    ]
    yzero = np.zeros((8 * N, DIM), np.float32)
    out = jax.block_until_ready(fn(*concat_in, yzero))
    return np.asarray(out).reshape(8, N, DIM)


def kernel(x, w_qkv, w_proj, b_proj):
    x = np.asarray(x, dtype=np.float32)
    w_qkv = np.asarray(w_qkv, dtype=np.float32)
    w_proj = np.asarray(w_proj, dtype=np.float32)
    b_proj = np.asarray(b_proj, dtype=np.float32)

    in_maps = _host_prep(x, w_qkv, w_proj)
    parts = _run(in_maps)

    y = np.empty((B, N, DIM), dtype=np.float32)
    for b in range(B):
        y[b] = parts[2 * b] + parts[2 * b + 1] + b_proj
    return y



# revision 59
# speedup vs baseline: 1.2595x; 1.0811x over previous
"""V11 Trainium2 Bass kernel for nn_Attention (B=4, N=2048, H=12, D=64).

Sharding: 8 cores = 4 batches x 2 head-groups of 6 heads. Per core: fused
qkv-projection + attention + output-projection, bf16 matmuls / fp32 psum.

Structure (V2 base): AV matmuls interleaved into the QK^T/exp chunk stream;
score chunks CH=2 ([128,2,512] psum, 2 banks); qkv/output projections emitted
as "filler" groups spread across chunk steps. Banks: scores 2x2, AV 2, misc 2.

V4/V5 changes (measured on HW via reps-differencing, body-iter 471us -> ~310us):
- Normalize chain: DVE `reciprocal` is 8 cyc/elem (~4.3us per [1,512]!);
  replaced with bit-trick seed + 1 Newton step (plain DVE TT/TS ops, max rel
  err 2.6e-3), and the PSUM copy + multiply fused into one
  scalar_tensor_tensor that reads the AV psum directly.  [-60us]
- AV split 10 bf16 / 6 fp8 kv-blocks: high blocks use fp8e4 DoubleRow
  (2 kv blocks per matmul; exp written as fp8 with bias -2). Full-fp8 fails
  the 2e-2 gate (2.5e-2); this split measures 1.59e-2.  [-25us]
- v_group inverted: x-block stationary, wv streams 384 wide (one matmul per
  contraction block for all 6 heads instead of three 128-wide ones). [-15us]

V8/V10: AV matmuls lag their exp by 2 chunks (never stall the PE FIFO on
ACT), and the slot tail (last two AV groups + both normalize chains, ~9us of serial
PE-then-DVE work) is deferred as ONE closure and emitted right after the
NEXT slot's first QK chunk. The next exp is then never queued behind the
boundary work, and emission stays read-before-write on the reused ps_av
banks (finer-grained deferral of individual chain ops corrupts results -
see make_chain comment). No filler pop at chunk 0, so QK(1) follows the
popped tail immediately in the PE FIFO. A/B-measured wins: V8 -12% vs V5;
lag-2 and the chunk-0 filler skip each ~-7..10% vs V8.

Known-flat: 2-strip/2-half weight-reuse restructures of qk_group/proj_group
regressed (no LDWEIGHTS dedup win; interleaved 2-bank accumulation hurts).
QK^T 64-row tile_position head-pairs DO run concurrently on HW.
"""

import sys

if "/opt/trn_rl_repo" not in sys.path:
    sys.path.insert(0, "/opt/trn_rl_repo")

import numpy as np
import ml_dtypes

import concourse.bacc as bacc
import concourse.mybir as mybir
import concourse.tile as tile

FP32 = mybir.dt.float32
BF16 = mybir.dt.bfloat16
FP8 = mybir.dt.float8e4
U32 = mybir.dt.uint32
AF = mybir.ActivationFunctionType
ALU = mybir.AluOpType
DR = mybir.MatmulPerfMode.DoubleRow
EXP_BIAS = -2.0  # exp(s-2): keeps values <=~40, inside fp8e4 range
RECIP_MAGIC = 0x7EF311C3  # fp32 reciprocal bit-trick seed (1 Newton iter after)
NHI = 6  # kv blocks (of 16) routed through fp8 DoubleRow AV; rest bf16

DIM = 768
HEAD_DIM = 64
SCALE = HEAD_DIM ** -0.5
B, N = 4, 2048
HG = 6
CC = DIM // 128
PAIRS = HG // 2
S = N // 512
J = N // 128
CH = 2                      # kv blocks per score chunk
NCHUNK = J // CH            # 8 chunks per (head, strip)

_CACHED = {}


def build_core_program(reps=0, ablate=(), bodies=1):
    nc = bacc.Bacc("TRN2", debug=False, target_bir_lowering=False, num_devices=1)

    xt_d = nc.dram_tensor("xt", [DIM, N], BF16, kind="ExternalInput")
    wqk_d = nc.dram_tensor("wqk", [DIM, DIM], BF16, kind="ExternalInput")
    wv_d = nc.dram_tensor("wv", [DIM, HG * 64], BF16, kind="ExternalInput")
    wp_d = nc.dram_tensor("wp", [HG * 64, DIM], BF16, kind="ExternalInput")
    y_d = nc.dram_tensor("y", [N, DIM], FP32, kind="ExternalOutput")

    with tile.TileContext(nc) as tc:
        with (
            tc.tile_pool(name="persist", bufs=1) as persist,
            tc.tile_pool(name="exps", bufs=22) as exps_pool,
            tc.tile_pool(name="attnt", bufs=5) as attnt_pool,
            tc.tile_pool(name="small", bufs=4) as small_pool,
            tc.tile_pool(name="ysb", bufs=3) as y_pool,
            tc.tile_pool(name="ps_score", bufs=2, space="PSUM") as ps_score,
            tc.tile_pool(name="ps_av", bufs=2, space="PSUM") as ps_av,
            tc.tile_pool(name="ps_misc", bufs=2, space="PSUM") as ps_misc,
        ):
            JLO = J - NHI       # low kv blocks: bf16 AV
            CLO = JLO // CH     # chunks below this index are bf16

            xT = persist.tile([128, CC, N], BF16)
            wqk = persist.tile([128, CC, DIM], BF16)
            wv = persist.tile([128, CC, HG * 64], BF16)
            wp = persist.tile([128, PAIRS, DIM], BF16)
            qkT = persist.tile([128, CC, N], BF16)
            vlo = persist.tile([128, JLO, HG, 65], BF16)
            # high blocks in fp8, paired for DoubleRow: [key, pair, plane, head, d]
            # d padded 65->80 so the plane stride (HG*80 = 480B) is 16B-aligned
            vhi = persist.tile([128, NHI // 2, 2, HG, 80], FP8)
            ebias = persist.tile([128, 1], FP32)
            nc.vector.memset(ebias, EXP_BIAS)
            magic = persist.tile([1, 512], U32)
            nc.vector.memset(magic, RECIP_MAGIC)

            xt_r = xt_d.ap().rearrange("(o p) n -> p o n", p=128)
            wqk_r = wqk_d.ap().rearrange("(o p) n -> p o n", p=128)
            wv_r = wv_d.ap().rearrange("(o p) n -> p o n", p=128)
            for cc in range(CC):
                nc.sync.dma_start(out=wqk[:, cc], in_=wqk_r[:, cc])
                nc.sync.dma_start(out=xT[:, cc], in_=xt_r[:, cc])
                nc.sync.dma_start(out=wv[:, cc], in_=wv_r[:, cc])
            nc.sync.dma_start(out=wp, in_=wp_d.ap().rearrange("(o p) n -> p o n", p=128))
            nc.vector.memset(vlo, 1.0)
            nc.vector.memset(vhi, 1.0)

            # ---------- filler groups (each: one psum-group of work) ----------

            def qk_group(ot, s):
                ps = ps_misc.tile([128, 512], FP32, tag="m")
                for cc in range(CC):
                    nc.tensor.matmul(
                        ps,
                        wqk[:, cc, ot * 128 : ot * 128 + 128],
                        xT[:, cc, s * 512 : s * 512 + 512],
                        start=(cc == 0), stop=(cc == CC - 1),
                    )
                nc.vector.tensor_copy(out=qkT[:, ot, s * 512 : s * 512 + 512], in_=ps)

            def v_group(nt):
                # all 6 heads at once: x-block stationary, wv streams 384 wide
                psv = ps_misc.tile([128, 512], FP32, tag="m")
                for cc in range(CC):
                    nc.tensor.matmul(
                        psv[:, 0 : HG * 64],
                        xT[:, cc, nt * 128 : nt * 128 + 128],
                        wv[:, cc, :],
                        start=(cc == 0), stop=(cc == CC - 1),
                    )
                for h in range(HG):
                    if nt < JLO:
                        vdst = vlo[:, nt, h, 0:64]
                    else:
                        m = nt - JLO
                        vdst = vhi[:, m // 2, m % 2, h, 0:64]
                    nc.vector.tensor_copy(
                        out=vdst,
                        in_=psv[:, h * 64 : h * 64 + 64],
                    )

            def proj_group(s, attnT, nt, og, ow, ysb):
                psy = ps_misc.tile([128, 512], FP32, tag="m")
                for cc in range(PAIRS):
                    nc.tensor.matmul(
                        psy[:, 0:ow],
                        attnT[:, cc, nt * 128 : nt * 128 + 128],
                        wp[:, cc, og : og + ow],
                        start=(cc == 0), stop=(cc == PAIRS - 1),
                    )
                nc.vector.tensor_copy(out=ysb[:, og : og + ow], in_=psy[:, 0:ow])
                if og == 512:
                    row = s * 512 + nt * 128
                    nc.sync.dma_start(out=y_d.ap()[row : row + 128, :], in_=ysb)

            def qkv_pair_fillers(p):
                out = []
                for ot in (p, PAIRS + p):
                    for s in range(S):
                        out.append(lambda ot=ot, s=s: qk_group(ot, s))
                return out

            def proj_fillers(s, attnT):
                if "fill" in ablate:
                    return []
                out = []
                for nt in range(4):
                    ysb = y_pool.tile([128, DIM], FP32, tag="y")
                    for og, ow in ((0, 512), (512, 256)):
                        out.append(
                            lambda nt=nt, og=og, ow=ow, ysb=ysb:
                                proj_group(s, attnT, nt, og, ow, ysb)
                        )
                return out

            def body():
                # lead-in: only qT/kT of pair 0; its v groups trickle into
                # slot (0,0)'s chunk stream (ACT starts ~18us earlier)
                for ot in (0, PAIRS):
                    for s in range(S):
                        qk_group(ot, s)

                filler = []
                pending = []   # deferred normalize-chain ops (DVE/Pool)
                attnT_tiles = {}

                def make_chain(pav, attnT, hp, h2):
                    # normalize: 1/denom via bit-trick + 1 Newton step on
                    # DVE (reciprocal op is 8 cyc/elem - far too slow), then
                    # Pool broadcast + one fused PSUM-read multiply into
                    # attnT. All ops run inline at slot end: deferring any of
                    # them into the next slot's chunk stream (to unblock the
                    # DVE queue) corrupts results - ps_av's 2 buffers are
                    # rewritten next slot, and even SBUF-only deferred ops
                    # came back with stale values. Returns no deferred ops.
                    if "div" in ablate:
                        return [lambda: nc.vector.memset(
                            attnT[h2 * 64 : h2 * 64 + 64, hp, 0:2], 0.5)]
                    den = pav[h2][64:65, :]
                    x0 = small_pool.tile([1, 512], FP32, tag="x0")
                    nc.vector.tensor_tensor(
                        x0[:, :].bitcast(U32), magic,
                        den.bitcast(U32), ALU.subtract,
                    )
                    t = small_pool.tile([1, 512], FP32, tag="t")
                    nc.vector.tensor_tensor(t, den, x0, ALU.mult)
                    u = small_pool.tile([1, 512], FP32, tag="u")
                    nc.vector.tensor_scalar(
                        out=u, in0=t, scalar1=-1.0, scalar2=2.0,
                        op0=ALU.mult, op1=ALU.add,
                    )
                    x1 = small_pool.tile([1, 512], FP32, tag="x1")
                    nc.vector.tensor_tensor(x1, x0, u, ALU.mult)
                    rb = small_pool.tile([64, 512], FP32, tag="rb")
                    nc.gpsimd.partition_broadcast(rb, x1, channels=64)
                    nc.vector.scalar_tensor_tensor(
                        out=attnT[h2 * 64 : h2 * 64 + 64, hp, :],
                        in0=pav[h2][0:64, :],
                        scalar=1.0,
                        in1=rb,
                        op0=ALU.mult,
                        op1=ALU.mult,
                    )
                    return []

                for hp in range(PAIRS):
                    if hp + 1 < PAIRS:
                        filler.extend(qkv_pair_fillers(hp + 1))
                    for s in range(S):
                        if hp == 0:
                            at = attnt_pool.tile(
                                [128, PAIRS, 512], BF16, tag="attnT",
                                name=f"attnT{s}",
                            )
                            attnT_tiles[s] = at
                        attnT = attnT_tiles[s]

                        pav = {}
                        for h2 in range(2):
                            pav[h2] = ps_av.tile(
                                [128, 512], FP32, tag="av", name=f"pav{h2}"
                            )

                        expS = {0: [None] * NCHUNK, 1: [None] * NCHUNK}

                        def emit_qkt_exp(c, s=s, hp=hp, expS=expS):
                            for h2, base in ((0, 0), (1, 64)):
                                pss = ps_score.tile(
                                    [128, CH, 512], FP32, tag="sc"
                                )
                                if "qkt" in ablate and "exp" not in ablate:
                                    nc.vector.memset(pss[:, 0, 0:2], 0.0)
                                for jj in range(CH):
                                    j = c * CH + jj
                                    if "qkt" in ablate:
                                        continue
                                    nc.tensor.matmul(
                                        pss[:, jj, :],
                                        qkT[base : base + 64, PAIRS + hp,
                                            j * 128 : j * 128 + 128],
                                        qkT[base : base + 64, hp,
                                            s * 512 : s * 512 + 512],
                                        start=True, stop=True,
                                        tile_position=(base, 0),
                                    )
                                et = exps_pool.tile(
                                    [128, CH, 512],
                                    BF16 if c < CLO else FP8, tag="e",
                                )
                                if "exp" not in ablate:
                                    nc.scalar.activation(
                                        out=et, in_=pss, func=AF.Exp,
                                        bias=ebias[:, :],
                                    )
                                else:
                                    nc.vector.memset(et[:, 0, 0:4], 1.0)
                                expS[h2][c] = et

                        def emit_av(c, hp=hp, pav=pav, expS=expS):
                            if "av" in ablate:
                                return
                            for h2 in range(2):
                                h = 2 * hp + h2
                                et = expS[h2][c]
                                if c < CLO:
                                    for jj in range(CH):
                                        j = c * CH + jj
                                        nc.tensor.matmul(
                                            pav[h2][0:65, :],
                                            vlo[:, j, h, :],
                                            et[:, jj, :],
                                            start=(j == 0), stop=False,
                                            skip_group_check=True,
                                        )
                                else:
                                    m = c - CLO
                                    nc.tensor.matmul(
                                        pav[h2][0:65, :],
                                        vhi[:, m, :, h, 0:65],
                                        et[:, :, :],
                                        start=False, stop=(c == NCHUNK - 1),
                                        perf_mode=DR,
                                        skip_group_check=True,
                                    )

                        first_slot = hp == 0 and s == 0
                        for c in range(NCHUNK):
                            emit_qkt_exp(c)
                            if c == 0 and pending:
                                # previous slot's tail (its last two AV
                                # groups + normalize chains) runs after this
                                # slot's first QK chunk: the next exp is
                                # never delayed behind boundary work, and the
                                # chains' pav reads and attnT writes are
                                # emitted before this slot's first pav write
                                # (c==2) and before any proj filler pop
                                pending.pop(0)()
                            if c > 0 and first_slot and "fill" not in ablate:
                                v_group(2 * (c - 1))
                                v_group(2 * (c - 1) + 1)
                            if c > 1:
                                # AV lags 2 chunks: by the time an AV matmul
                                # reaches the PE FIFO head its exp input is
                                # long finished, so it never stalls the queue
                                emit_av(c - 2)
                            if c > 0 and filler and not first_slot:
                                # no filler at chunk 0: keeps QK(1) right
                                # behind the popped slot tail in the PE FIFO
                                filler.pop(0)()
                        if first_slot and "fill" not in ablate:
                            v_group(14)
                            v_group(15)

                        def slot_tail(pav=pav, attnT=attnT, hp=hp,
                                      emit_av=emit_av):
                            emit_av(NCHUNK - 2)
                            emit_av(NCHUNK - 1)
                            if "av" in ablate:
                                for h2 in range(2):
                                    nc.vector.memset(pav[h2][0:1, 0:2], 1.0)
                            for h2 in range(2):
                                make_chain(pav, attnT, hp, h2)

                        pending.append(slot_tail)

                        if hp == PAIRS - 1:
                            filler.extend(
                                proj_fillers(s, attnT_tiles.pop(s))
                            )

                # drain the last slot's tail, then remaining fillers (tail
                # projections - they depend on the last chains' attnT)
                for op in pending:
                    op()
                for f in filler:
                    f()

            from contextlib import nullcontext
            with (tc.For_i(0, reps, 1) if reps else nullcontext()):
                for _ in range(bodies):
                    body()

    nc.compile()
    return nc


def _host_prep(x, w_qkv, w_proj):
    bf16 = ml_dtypes.bfloat16
    in_maps = []
    for c in range(8):
        b, hg = c // 2, c % 2
        r0 = 384 * hg
        wq = w_qkv[r0 : r0 + 384] * SCALE
        wk = w_qkv[768 + r0 : 768 + r0 + 384]
        wvv = w_qkv[1536 + r0 : 1536 + r0 + 384]
        wqk = np.concatenate([wq, wk], axis=0)
        in_maps.append({
            "xt": np.ascontiguousarray(x[b].T).astype(bf16),
            "wqk": np.ascontiguousarray(wqk.T).astype(bf16),
            "wv": np.ascontiguousarray(wvv.T).astype(bf16),
            "wp": np.ascontiguousarray(w_proj[:, r0 : r0 + 384].T).astype(bf16),
        })
    return in_maps


def _get_fn():
    if "fn" in _CACHED:
        return _CACHED["fn"]

    import jax
    from jax.sharding import Mesh, PartitionSpec
    from jax.experimental.shard_map import shard_map
    from concourse import bass2jax
    from concourse.bass2jax import _bass_exec_p, install_neuronx_cc_hook

    install_neuronx_cc_hook()
    nc = build_core_program()

    in_names = ["xt", "wqk", "wv", "wp"]
    out_avals = [jax.core.ShapedArray((N, DIM), np.float32)]
    partition_name = nc.partition_id_tensor.name if nc.partition_id_tensor else None

    def _body(xt, wqk, wvv, wp, yzero):
        operands = [xt, wqk, wvv, wp, yzero]
        names = in_names + ["y"]
        if nc.dbg_addr is not None:
            operands.append(np.zeros((1, 2), np.uint32))
            names.append(nc.dbg_addr.name)
        if partition_name is not None:
            operands.append(bass2jax.partition_id_tensor())
            names.append(partition_name)
        outs = _bass_exec_p.bind(
            *operands,
            out_avals=tuple(out_avals),
            in_names=tuple(names),
            out_names=("y",),
            lowering_input_output_aliases=(),
            sim_require_finite=True,
            sim_require_nnan=True,
            nc=nc,
        )
        return outs[0]

    devices = jax.devices()[:8]
    mesh = Mesh(np.asarray(devices), ("core",))
    fn = jax.jit(
        shard_map(
            _body, mesh=mesh,
            in_specs=(PartitionSpec("core"),) * 5,
            out_specs=PartitionSpec("core"),
            check_rep=False,
        ),
        keep_unused=True,
    )
    _CACHED["fn"] = fn
    return fn


def _run(in_maps):
    import jax

    fn = _get_fn()
    concat_in = [
        np.concatenate([m[name] for m in in_maps], axis=0)
        for name in ["xt", "wqk", "wv", "wp"]
    ]
    yzero = np.zeros((8 * N, DIM), np.float32)
    out = jax.block_until_ready(fn(*concat_in, yzero))
    return np.asarray(out).reshape(8, N, DIM)


def kernel(x, w_qkv, w_proj, b_proj):
    x = np.asarray(x, dtype=np.float32)
    w_qkv = np.asarray(w_qkv, dtype=np.float32)
    w_proj = np.asarray(w_proj, dtype=np.float32)
    b_proj = np.asarray(b_proj, dtype=np.float32)

    in_maps = _host_prep(x, w_qkv, w_proj)
    parts = _run(in_maps)

    y = np.empty((B, N, DIM), dtype=np.float32)
    for b in range(B):
        y[b] = parts[2 * b] + parts[2 * b + 1] + b_proj
    return y

